# revision 36
# baseline (speedup 1.0000x reference)
"""Trainium2 Bass kernel for BEiT-style dot-product attention with relative
position bias (batch 8, seq 1025, dim 1024, 16 heads).

Strategy: data-parallel — one batch element per NeuronCore (8 cores).
V2: all-fp16 matmuls (1 cyc/row on PE vs ~1.5 for fp32r), per-j-tile
attention pipeline with both heads of a pair sharing one 2-bank PSUM score
tile, k-bias dropped (softmax-invariant), no GpSimd/SWDGE anywhere, and
full-width reciprocal after an HWDGE broadcast.

Layouts (per core):
  xt   [1025, 1152] fp16 : x[b].T padded to seq 1152 (9 j-tiles), plus a ones
                           row (row 1024) folding the qkv bias into K=1 matmuls.
  q    computed transposed [chan, seq] with +bq; k transposed, NO +bk (adding
       bk shifts scores by a per-i constant over j -> softmax invariant).
  v    natural [seq, chan] per head with a ones column ([v_h | 1]) so the PV
       matmul (M=65) emits the softmax denominator as psum row 64 for free.
  scores S.T [j, i]: per (head-pair p, j-tile) one [128, 2, 512] psum tile
       (bank per head); QK (K=64, tile_position packed); ScalarE exp
       (scale=1/8) -> fp16, then VectorE multiply by exp(bias) (multiplicative
       rel-pos bias: exp(s+b) = exp(s)*exp(b); pad keys get eb=0) -> fp16 e
       tiles; PV accumulates over the 9 j-tiles into [65, 512] psum per head.
  norm: denominator rows copied to SBUF by ScalarE, HWDGE round-trip
       broadcast to 128 partitions, VectorE reciprocal + multiply.
  proj y = outT.T @ proj_w.T + proj_b (ones-row K=1 trick again).
"""

import os
import sys

for _p in (
    "/root/.axon_site",
    "/root/.axon_site/_ro/trn_rl_repo",
    "/root/.axon_site/_ro/pypackages",
    "/opt/trn_rl_repo",
    "/opt/pypackages",
):
    if os.path.isdir(_p) and _p not in sys.path:
        sys.path.append(_p)

import numpy as np

import concourse.bass as bass
import concourse.bacc as bacc
import concourse.tile as tile
import concourse.mybir as mybir
from concourse.bass_utils import run_bass_kernel_spmd

F32 = mybir.dt.float32
F16 = mybir.dt.float16
EXPFN = mybir.ActivationFunctionType.Exp
COPYFN = mybir.ActivationFunctionType.Copy

SEQ = 1025          # 32*32 grid + 1 cls token
SP = 1152           # padded seq (9 j-tiles of 128)
D = 1024
H = 16
NB = 8              # batch == cores
NJT = SP // 128     # 9
IB = [(0, 512), (512, 512)]                  # full i-blocks; i=1024 special
QB = [(0, 512), (512, 512), (1023, 2)]       # q-production i-blocks
KB = [(0, 512), (512, 512), (1024, 128)]     # k-production j-blocks
CBV = [(0, 512), (512, 512)]                 # v-production channel blocks
FB = [(0, 512), (512, 512)]                  # proj output-channel blocks
NEG = -60000.0                               # pad-key bias (fp16-safe, exp->0)

_CACHE = {}


def _build_module():
    nc = bacc.Bacc()
    xt_d = nc.dram_tensor("xt", [SEQ, SP], F16, kind="ExternalInput")
    wqk_d = nc.dram_tensor("wqk", [SEQ, 2 * D], F16, kind="ExternalInput")
    wv_d = nc.dram_tensor("wv", [SEQ, D], F16, kind="ExternalInput")
    wp_d = nc.dram_tensor("wp", [SEQ, D], F16, kind="ExternalInput")
    bias_d = nc.dram_tensor("biasq", [2, 8, SP, 1024], F16, kind="ExternalInput")
    bias1_d = nc.dram_tensor("bias1", [H, SP, 2], F16, kind="ExternalInput")
    y_d = nc.dram_tensor("y", [SEQ, D], F32, kind="ExternalOutput")

    with tile.TileContext(nc) as tc:
        with (
            tc.tile_pool(name="persist", bufs=1) as pp,
            tc.tile_pool(name="consts", bufs=1) as cp,
        ):
            qt = pp.tile([128, 8, SEQ], F16, tag="qt")
            kt = pp.tile([128, 8, SP], F16, tag="kt")
            va = pp.tile([128, NJT, H, 65], F16, tag="va")
            out1 = pp.tile([128, 8, 1], F16, tag="out1")

            ones_col_f = cp.tile([128, NJT * H], F32, tag="onescolf")
            ones_row_f = cp.tile([1, 512], F32, tag="onesrowf")
            ones_row = cp.tile([1, 512], F16, tag="onesrow")
            wql = cp.tile([1, D], F16, tag="wql")
            wvl = cp.tile([1, D], F16, tag="wvl")
            wpl = cp.tile([1, D], F16, tag="wpl")
            nc.sync.dma_start(out=wql, in_=wqk_d[SEQ - 1 : SEQ, 0:D])
            nc.sync.dma_start(out=wvl, in_=wv_d[SEQ - 1 : SEQ, :])
            nc.sync.dma_start(out=wpl, in_=wp_d[SEQ - 1 : SEQ, :])
            nc.vector.memset(ones_col_f, 1.0)
            nc.vector.memset(ones_row_f, 1.0)
            nc.vector.tensor_copy(ones_row, ones_row_f)
            # ones columns of v_aug ([128, 9, 16, 1] strided view)
            nc.vector.tensor_copy(
                va[:, :, :, 64:65],
                ones_col_f.rearrange("p (t h) -> p t h", t=NJT).unsqueeze(3),
            )

            # ---------------- Phase A: projections ----------------
            with (
                tc.tile_pool(name="xa", bufs=1) as xa,
                tc.tile_pool(name="wload", bufs=3) as wl,
                tc.tile_pool(name="psA", bufs=6, space="PSUM") as psA,
            ):
                xt = xa.tile([128, 8, SP], F16, tag="xt")
                xtl = xa.tile([1, SP], F16, tag="xtl")
                for ec in range(8):
                    nc.sync.dma_start(
                        out=xt[:, ec, :],
                        in_=xt_d[ec * 128 : (ec + 1) * 128, :],
                    )
                nc.sync.dma_start(out=xtl, in_=xt_d[D : D + 1, :])

                # Q (channels 0:1024, +bias) and K (1024:2048, no bias),
                # transposed layout
                for ct in range(16):
                    w = wl.tile([128, 8, 128], F16, tag="wqk")
                    nc.sync.dma_start(
                        out=w,
                        in_=wqk_d[0:D, ct * 128 : (ct + 1) * 128].rearrange(
                            "(c p) m -> p c m", p=128
                        ),
                    )
                    blocks = QB if ct < 8 else KB
                    # keep each weight chunk stationary across all i-blocks
                    pas = [psA.tile([128, 512], F32, tag="psA", name=f"paqk{bi}") for bi in range(len(blocks))]
                    for ec in range(8):
                        for bi, (i0, iw) in enumerate(blocks):
                            nc.tensor.matmul(
                                pas[bi][:, :iw],
                                w[:, ec, :],
                                xt[:, ec, i0 : i0 + iw],
                                start=(ec == 0),
                                stop=(ec == 7 if ct >= 8 else False),
                                skip_group_check=True,
                            )
                    for bi, (i0, iw) in enumerate(blocks):
                        if ct < 8:
                            nc.tensor.matmul(
                                pas[bi][:, :iw],
                                wql[0:1, ct * 128 : (ct + 1) * 128],
                                xtl[0:1, i0 : i0 + iw],
                                start=False,
                                stop=True,
                                skip_group_check=True,
                            )
                        dst = qt if ct < 8 else kt
                        nc.vector.tensor_copy(
                            dst[:, ct % 8, i0 : i0 + iw], pas[bi][:, :iw]
                        )

                # V, natural layout, with bias via ones-row K=1
                for cbi, (c0, cw) in enumerate(CBV):
                    wv = wl.tile([128, 8, 512], F16, tag="wv")
                    nc.sync.dma_start(
                        out=wv[:, :, :cw],
                        in_=wv_d[0:D, c0 : c0 + cw].rearrange(
                            "(c p) m -> p c m", p=128
                        ),
                    )
                    for g0, gn in ((0, 5), (5, 4)):
                        pas = [psA.tile([128, 512], F32, tag="psA", name=f"pav{gi}") for gi in range(gn)]
                        for ec in range(8):
                            for gi in range(gn):
                                jt = g0 + gi
                                nc.tensor.matmul(
                                    pas[gi][:, :cw],
                                    xt[:, ec, jt * 128 : (jt + 1) * 128],
                                    wv[:, ec, :cw],
                                    start=(ec == 0),
                                    stop=False,
                                    skip_group_check=True,
                                )
                        for gi in range(gn):
                            jt = g0 + gi
                            nc.tensor.matmul(
                                pas[gi][:, :cw],
                                xtl[0:1, jt * 128 : (jt + 1) * 128],
                                wvl[0:1, c0 : c0 + cw],
                                start=False,
                                stop=True,
                                skip_group_check=True,
                            )
                            h0 = c0 // 64
                            nh = cw // 64
                            nc.vector.tensor_copy(
                                va[:, jt, h0 : h0 + nh, 0:64],
                                pas[gi][:, :cw].rearrange("p (h c) -> p h c", c=64),
                            )

            # ---------------- Phase B: attention + proj ----------------
            with (
                tc.tile_pool(name="biasp", bufs=18) as bp,
                tc.tile_pool(name="attnp", bufs=4) as ap,
                tc.tile_pool(name="normp", bufs=2) as rp,
                tc.tile_pool(name="outp", bufs=1) as op,
                tc.tile_pool(name="projw", bufs=2) as pw,
                tc.tile_pool(name="yp", bufs=1) as yp,
                tc.tile_pool(name="dramp", bufs=2, space="DRAM") as dp,
                tc.tile_pool(name="psS", bufs=3, space="PSUM") as psS,
                tc.tile_pool(name="psPV", bufs=1, space="PSUM") as psPV,
            ):
                # width-1 column (i=1024): per-p attention pass writing
                # unnormalized out1 + denominators into a shared tile; one
                # batched broadcast round-trip normalizes all 16 heads at once
                def emit_width1_pass(p, dnw):
                    h0, h1 = 2 * p, 2 * p + 1
                    sx = psS.tile([128, 2, 512], F32, tag="S")
                    for jt in range(NJT):
                        js = slice(jt * 128, (jt + 1) * 128)
                        nc.tensor.matmul(
                            sx[:, 0, 2 * jt : 2 * jt + 2],
                            kt[0:64, p, js],
                            qt[0:64, p, 1023:1025],
                            start=True, stop=True,
                            skip_group_check=True,
                            tile_position=(0, 0),
                        )
                        nc.tensor.matmul(
                            sx[:, 1, 2 * jt : 2 * jt + 2],
                            kt[64:128, p, js],
                            qt[64:128, p, 1023:1025],
                            start=True, stop=True,
                            skip_group_check=True,
                            tile_position=(64, 0),
                        )
                    b1 = rp.tile([128, 2, NJT, 2], F16, tag="b1w")
                    for hh, hcur in enumerate((h0, h1)):
                        nc.sync.dma_start(
                            out=b1[:, hh, :, :],
                            in_=bias1_d[hcur, :, :].rearrange("(t p) i -> p t i", p=128),
                        )
                    e1r = ap.tile([128, 2, 2 * NJT], F16, tag="e1r")
                    nc.scalar.activation(e1r[:, 0, :], sx[:, 0, 0 : 2 * NJT], EXPFN, scale=0.125)
                    nc.scalar.activation(e1r[:, 1, :], sx[:, 1, 0 : 2 * NJT], EXPFN, scale=0.125)
                    e1x = ap.tile([128, 2, 2 * NJT], F16, tag="e1x")
                    nc.vector.tensor_mul(
                        e1x, e1r,
                        b1.rearrange("p h t i -> p h (t i)"),
                    )
                    pv0 = psPV.tile([128, 512], F32, tag="pv0")
                    pv1 = psPV.tile([128, 512], F32, tag="pv1")
                    for jt in range(NJT):
                        nc.tensor.matmul(
                            pv0[0:65, 0:1], va[:, jt, h0, :], e1x[:, 0, 2 * jt + 1 : 2 * jt + 2],
                            start=(jt == 0), stop=(jt == NJT - 1), skip_group_check=True,
                        )
                        nc.tensor.matmul(
                            pv1[0:65, 0:1], va[:, jt, h1, :], e1x[:, 1, 2 * jt + 1 : 2 * jt + 2],
                            start=(jt == 0), stop=(jt == NJT - 1), skip_group_check=True,
                        )
                    nc.scalar.activation(dnw[64:65, 0, p : p + 1], pv0[64:65, 0:1], COPYFN)
                    nc.scalar.activation(dnw[64:65, 1, p : p + 1], pv1[64:65, 0:1], COPYFN)
                    nc.vector.tensor_copy(out1[0:64, p, :], pv0[0:64, 0:1])
                    nc.vector.tensor_copy(out1[64:128, p, :], pv1[0:64, 0:1])

                def emit_width1_norm(dnw):
                    rdr1 = dp.tile([2, 8], F32, tag="rdr1")
                    nc.sync.dma_start(out=rdr1, in_=dnw[64:65, :, :])
                    bc1 = rp.tile([128, 8], F32, tag="bc1")
                    nc.sync.dma_start(
                        out=bc1,
                        in_=bass.AP(tensor=rdr1.tensor, offset=rdr1.offset,
                                    ap=[[8, 2], [0, 64], [1, 8]]),
                    )
                    rec1 = rp.tile([128, 8], F32, tag="rec1")
                    nc.vector.reciprocal_approx_fast(out=rec1, in_=bc1)
                    nc.vector.tensor_mul(out1[:, :, 0], out1[:, :, 0], rec1)

                # ---- main i-blocks ----
                for ib, (i0, iw) in enumerate(IB):
                    out_all = op.tile([128, 8, 512], F16, tag="out_all")
                    rdr_ib = dp.tile([8, 2, 512], F32, tag="rdr_ib")
                    bcs = []
                    for p in range(8):
                        h0, h1 = 2 * p, 2 * p + 1
                        pv0 = psPV.tile([128, 512], F32, tag="pv0")
                        pv1 = psPV.tile([128, 512], F32, tag="pv1")
                        pend = None
                        for jt in range(NJT + 1):
                            if jt < NJT:
                                js = slice(jt * 128, (jt + 1) * 128)
                                b = bp.tile([128, 2, 512], F16, tag="b")
                                nc.sync.dma_start(
                                    out=b,
                                    in_=bias_d[ib, p, jt * 128 : (jt + 1) * 128, :],
                                )
                                s = psS.tile([128, 2, 512], F32, tag="S")
                                nc.tensor.matmul(
                                    s[:, 0, :], kt[0:64, p, js], qt[0:64, p, i0 : i0 + iw],
                                    start=True, stop=True, skip_group_check=True,
                                    tile_position=(0, 0),
                                )
                                nc.tensor.matmul(
                                    s[:, 1, :], kt[64:128, p, js], qt[64:128, p, i0 : i0 + iw],
                                    start=True, stop=True, skip_group_check=True,
                                    tile_position=(64, 0),
                                )
                                er = ap.tile([128, 2, 512], F16, tag="er")
                                nc.scalar.activation(er, s, EXPFN, scale=0.125)
                                e = ap.tile([128, 2, 512], F16, tag="e")
                                # split the eb multiplies between DVE and GpSimd
                                eng = nc.gpsimd if jt % 3 == 2 else nc.vector
                                eng.tensor_mul(e, er, b)
                            # PV for the previous j-tile (software pipelined)
                            if pend is not None:
                                pe, pjt = pend
                                nc.tensor.matmul(
                                    pv0[0:65, :], va[:, pjt, h0, :], pe[:, 0, :],
                                    start=(pjt == 0), stop=(pjt == NJT - 1),
                                    skip_group_check=True,
                                )
                                nc.tensor.matmul(
                                    pv1[0:65, :], va[:, pjt, h1, :], pe[:, 1, :],
                                    start=(pjt == 0), stop=(pjt == NJT - 1),
                                    skip_group_check=True,
                                )
                            if jt < NJT:
                                pend = (e, jt)

                        # drain pv to SBUF unnormalized (frees psum banks fast);
                        # denominators go to DRAM per p, and the whole ib is
                        # normalized in one batch at the end (keeps the slow
                        # broadcast round-trip out of the DVE FIFO)
                        dn = rp.tile([128, 2, 512], F32, tag="dn")
                        nc.scalar.activation(dn[64:65, 0, :], pv0[64:65, :], COPYFN)
                        nc.scalar.activation(dn[64:65, 1, :], pv1[64:65, :], COPYFN)
                        nc.vector.tensor_copy(out_all[0:64, p, :], pv0[0:64, :])
                        nc.vector.tensor_copy(out_all[64:128, p, :], pv1[0:64, :])
                        nc.sync.dma_start(out=rdr_ib[p, :, :], in_=dn[64:65, :, :])
                        # kick off the broadcast round-trip now; recip+mul
                        # happen at ib end (keeps slow DMA out of DVE FIFO)
                        bc = rp.tile([128, 512], F32, tag="bc", bufs=8, name=f"bc{p}")
                        nc.sync.dma_start(
                            out=bc,
                            in_=bass.AP(tensor=rdr_ib.tensor,
                                        offset=rdr_ib.offset + p * 1024,
                                        ap=[[512, 2], [0, 64], [1, 512]]),
                        )
                        bcs.append(bc)

                    # per-pair normalization at ib end; all broadcasts are
                    # already in flight, so these run back-to-back
                    for p in range(8):
                        rec = rp.tile([128, 512], F32, tag="rec", name=f"rec{p}")
                        nc.vector.reciprocal_approx_fast(out=rec, in_=bcs[p])
                        nc.vector.tensor_mul(out_all[:, p, :], out_all[:, p, :], rec)

                    if ib == 0:
                        dnw = rp.tile([128, 2, 8], F32, tag="dnw")
                        for p in range(8):
                            emit_width1_pass(p, dnw)
                        emit_width1_norm(dnw)

                    # proj for this i-block
                    ysb = yp.tile([128, 4, D], F32, tag="ysb")
                    y1 = yp.tile([1, D], F32, tag="y1")
                    wpjs = []
                    for f0, fw in FB:
                        wpj = pw.tile([128, 8, 512], F16, tag="wpj")
                        nc.sync.dma_start(
                            out=wpj[:, :, :fw],
                            in_=wp_d[0:D, f0 : f0 + fw].rearrange(
                                "(c p) m -> p c m", p=128
                            ),
                        )
                        wpjs.append(wpj)
                    for (f0, fw), wpj in zip(FB, wpjs):
                        # cc-outer: accumulate all 4 i-tiles per cc chunk so the
                        # first matmuls only need head pair 0 normalized
                        pjA = psS.tile([128, 2, 512], F32, tag="S", name=f"pjA{f0}")
                        pjB = psS.tile([128, 2, 512], F32, tag="S", name=f"pjB{f0}")
                        pj = [pjA[:, 0, :], pjA[:, 1, :], pjB[:, 0, :], pjB[:, 1, :]]
                        for cc in range(8):
                            for it in range(4):
                                nc.tensor.matmul(
                                    pj[it][:, :fw],
                                    out_all[:, cc, it * 128 : (it + 1) * 128],
                                    wpj[:, cc, :fw],
                                    start=(cc == 0), stop=False,
                                    skip_group_check=True,
                                )
                        for it in range(4):
                            nc.tensor.matmul(
                                pj[it][:, :fw], ones_row[0:1, 0:128], wpl[0:1, f0 : f0 + fw],
                                start=False, stop=True, skip_group_check=True,
                            )
                            nc.vector.tensor_copy(ysb[:, it, f0 : f0 + fw], pj[it][:, :fw])
                        if ib == 1:
                            # the single i=1024 row rides the second block's weights
                            pj1 = psPV.tile([128, 512], F32, tag="pv1")
                            for cc in range(8):
                                nc.tensor.matmul(
                                    pj1[0:1, :fw], out1[:, cc, 0:1], wpj[:, cc, :fw],
                                    start=(cc == 0), stop=False, skip_group_check=True,
                                )
                            nc.tensor.matmul(
                                pj1[0:1, :fw], ones_row[0:1, 0:1], wpl[0:1, f0 : f0 + fw],
                                start=False, stop=True, skip_group_check=True,
                            )
                            nc.vector.tensor_copy(y1[0:1, f0 : f0 + fw], pj1[0:1, :fw])
                    for it in range(4):
                        nc.sync.dma_start(
                            out=y_d[i0 + it * 128 : i0 + (it + 1) * 128, :],
                            in_=ysb[:, it, :],
                        )
                    if ib == 1:
                        nc.sync.dma_start(out=y_d[1024:1025, :], in_=y1)

    nc.finalize()
    return nc


def _prepare_inputs(x, qkv_w, qkv_b, proj_w, proj_b, rel_pos_table, rel_pos_idx):
    """Host-side sharding/layout prep. Returns per-core input maps."""
    xf = np.asarray(x, dtype=np.float32)
    wqkv_aug = np.empty((SEQ, 3 * D), np.float32)
    wqkv_aug[0:D] = np.asarray(qkv_w, np.float32).T
    wqkv_aug[D] = np.asarray(qkv_b, np.float32)
    wqk = np.ascontiguousarray(wqkv_aug[:, 0 : 2 * D]).astype(np.float16)
    wv = np.ascontiguousarray(wqkv_aug[:, 2 * D : 3 * D]).astype(np.float16)

    wp = np.empty((SEQ, D), np.float32)
    wp[0:D] = np.asarray(proj_w, np.float32).T
    wp[D] = np.asarray(proj_b, np.float32)
    wp = wp.astype(np.float16)

    # multiplicative rel-pos bias: eb = exp(table[idx]); pad keys -> 0
    table = np.asarray(rel_pos_table, np.float32)               # [ndist, H]
    idx = np.asarray(rel_pos_idx)
    g = np.exp(table[idx])                                      # [i, j, H]
    ebT = np.zeros((H, SP, SEQ), np.float16)
    ebT[:, 0:SEQ, :] = g.transpose(2, 1, 0)                     # [H, j, i]
    bias1 = np.ascontiguousarray(ebT[:, :, SEQ - 2 : SEQ])      # [H, SP, 2]
    # [ib, head-pair, j, h-of-pair*512 + i'] with 2KB contiguous runs
    biasq = np.empty((2, 8, SP, 2, 512), np.float16)
    for ib in range(2):
        for pr in range(8):
            biasq[ib, pr, :, 0, :] = ebT[2 * pr, :, ib * 512 : (ib + 1) * 512]
            biasq[ib, pr, :, 1, :] = ebT[2 * pr + 1, :, ib * 512 : (ib + 1) * 512]
    biasq = biasq.reshape(2, 8, SP, 1024)

    in_maps = []
    for b in range(NB):
        xt = np.zeros((SEQ, SP), np.float16)
        xt[0:D, 0:SEQ] = xf[b].T.astype(np.float16)
        xt[D, 0:SEQ] = 1.0
        in_maps.append(
            {
                "xt": xt, "wqk": wqk, "wv": wv, "wp": wp,
                "biasq": biasq, "bias1": bias1,
            }
        )
    return in_maps


def run(inputs, trace=False):
    """Compile (cached) + run on 8 cores. Returns (out [8,1025,1024], results)."""
    if "nc" not in _CACHE:
        _CACHE["nc"] = _build_module()
    nc = _CACHE["nc"]
    in_maps = _prepare_inputs(**inputs)
    res = run_bass_kernel_spmd(
        nc, in_maps, core_ids=list(range(NB)), trace=trace,
        trace_cores=[0] if trace else None,
    )
    out = np.stack([res.results[b]["y"] for b in range(NB)], axis=0)
    return out, res


def kernel(**inputs) -> np.ndarray:
    out, _ = run(inputs, trace=False)
    return out
